# Initial kernel scaffold
#
"""Bayesian linear layer (reparameterized per-sample weights) on 8 trn2 NeuronCores.

y[b,o] = sum_i x[b,i] * (mu[o,i] + softplus(rho[o,i]) * eps_w[b,o,i])
         + bias_mu[o] + softplus(bias_rho[o]) * eps_b[b,o]

Sharding: data-parallel over batch. 8 cores x 32 samples. mu/rho replicated.

Per-core pipeline (v3-final).  The 128 MiB eps_w shard read dominates
(~426 GB/s measured on the SWDGE cast path); all big loads ride ONE SWDGE
queue in FIFO order (a parallel queue steals HBM read bandwidth from the
eps stream -- measured), and everything else hides under it:
  1. SWDGE DMA casts eps_w[b] fp32->bf16 on the way in, "(p c)" o-layout so
     each partition reads one contiguous 32 KiB run per sample.
  2. DVE single 2x-mode pass u = eps (*) sigma in natural layout (all-SBUF
     bf16 -> 2 elem/cycle/lane).
  3. PE transposes u in bf16 (transpose-mode; pair cost is LDWEIGHTS-bound
     at ~107 ns) into per-k [128, 1024] bf16 PSUM tiles (1 bank each).
  4. Act (5/8) + DVE (3/8) evacuate PSUM -> SBUF bf16 per k-chunk (bf16
     PSUM keeps the DVE copy in 2x mode).
  5. PE reduce-matmul, stationary = x[b, i_chunk] bf16 column (m=1),
     accumulates y2[b, half] over the 8 i-chunks in PSUM.
  6. y_mu + bias terms precomputed into C (f-ordered); per-sample y2 rows
     added via SBUF->SBUF accumulating DMA; one final unpermute + store.
No barriers: eps streaming starts at t=0 and setup hides under it.

Measured on 8xtrn2 (max over cores): 450 us with y2 double-buffered
(pt_bufs=3/ptf=1/y2_bufs=2; the single-buffered variant measured 477-482
back-to-back, 411 in an earlier favorable device state) vs 537 us for the
fp32 transpose-mode baseline; HBM-stream floor is ~340 us.
"""

import numpy as np

import concourse.bass as bass
from concourse import bacc
import concourse.mybir as mybir
import concourse.tile as tile
from concourse.bass import ts
from concourse.bass_utils import run_bass_kernel_spmd
from concourse.masks import make_identity

FP32 = mybir.dt.float32
BF16 = mybir.dt.bfloat16
AF = mybir.ActivationFunctionType

F = 1024          # feature dim (in == out)
N_CORES = 8
B_FULL = 256
NCH = F // 128    # 8 chunks of 128


def build_nc(BL: int, eps_bufs=5, u_bufs=2, ut_bufs=4, pt_bufs=3, y2_bufs=2,
             evac_dve=3) -> bass.Bass:
    """Build the per-core Bass program for a local batch of BL samples.

    o-index layout: o = 8*p + c (partition p in 0..127, chunk c in 0..7), so
    a partition's 8 o-rows are contiguous in HBM.  Column order after the PE
    transpose ("f-order"): f = c*128 + p.  C and y2 are kept f-ordered until
    a single strided pass at the end restores natural o order.
    """
    nc = bacc.Bacc(None, target_bir_lowering=False)

    x_d = nc.declare_dram_parameter("x", [BL, F], FP32, isOutput=False)
    mu_d = nc.declare_dram_parameter("weight_mu", [F, F], FP32, isOutput=False)
    rho_d = nc.declare_dram_parameter("weight_rho", [F, F], FP32, isOutput=False)
    bmu_d = nc.declare_dram_parameter("bias_mu", [F], FP32, isOutput=False)
    brho_d = nc.declare_dram_parameter("bias_rho", [F], FP32, isOutput=False)
    epsw_d = nc.declare_dram_parameter("eps_w", [BL, F, F], FP32, isOutput=False)
    epsb_d = nc.declare_dram_parameter("eps_b", [BL, F], FP32, isOutput=False)
    y_d = nc.declare_dram_parameter("y", [BL, F], FP32, isOutput=True)

    # o = 8p + c: partition p covers o in [8p, 8p+8) -> 32 KiB contiguous.
    epsw_t = epsw_d[:].rearrange("b (p c) i -> b p c i", p=128)
    mu_t = mu_d[:].rearrange("(p c) i -> p c i", p=128)
    rho_t = rho_d[:].rearrange("(p c) i -> p c i", p=128)

    with tile.TileContext(nc) as tc:
        with (
            tc.tile_pool(name="persist", bufs=1) as persist,
            tc.tile_pool(name="setup", bufs=1) as setupp,
            tc.tile_pool(name="eps", bufs=eps_bufs) as epsp,
            tc.tile_pool(name="u", bufs=u_bufs) as up,
            tc.tile_pool(name="ut", bufs=ut_bufs) as utp,
            tc.tile_pool(name="yrow", bufs=2) as yrowp,
            tc.tile_pool(name="pt", bufs=pt_bufs, space="PSUM") as ptp,
            tc.tile_pool(name="ptf", bufs=1, space="PSUM") as ptfp,
            tc.tile_pool(name="py2", bufs=y2_bufs, space="PSUM") as py2p,
        ):
            # ---------------- setup (overlaps with eps streaming) ----------
            ident = persist.tile([128, 128], BF16)
            make_identity(nc, ident)

            # sigma in natural (p c) layout, bf16
            rho_s = setupp.tile([128, NCH, F], BF16, tag="stage", name="rho_s")
            nc.gpsimd.dma_start(out=rho_s, in_=rho_t)
            sig = persist.tile([128, NCH, F], BF16)
            # softplus(x) = ln(1 + exp(x)); rho <= ~0 so no overflow
            nc.scalar.activation(out=sig, in_=rho_s, func=AF.Exp)
            nc.scalar.activation(out=sig, in_=sig, func=AF.Ln, bias=1.0)

            # first eps DMAs issue here (program order on the SWDGE queue:
            # rho, then eps[0..1], then mu, ...)
            eps_tiles: dict[int, object] = {}

            def eps_dma(b):
                if b >= BL or b in eps_tiles:
                    return
                eb = epsp.tile([128, NCH, F], BF16, tag="epst", name=f"eb{b}")
                nc.gpsimd.dma_start(out=eb, in_=epsw_t[b])
                eps_tiles[b] = eb

            eps_dma(0)
            eps_dma(1)

            # muT (bf16): stage in (p c) layout, transpose on PE, evac via Act
            mu_s = setupp.tile([128, NCH, F], BF16, tag="stage", name="mu_s")
            nc.gpsimd.dma_start(out=mu_s, in_=mu_t)
            muT = persist.tile([128, NCH, F], BF16)
            for k in range(NCH):
                pt_k = ptfp.tile([128, F], BF16, tag="ptf", name=f"ptmu{k}")
                for c in range(NCH):
                    nc.tensor.transpose(
                        out=pt_k[:, ts(c, 128)],
                        in_=mu_s[:, c, ts(k, 128)],
                        identity=ident,
                    )
                nc.scalar.copy(out=muT[:, k, :], in_=pt_k)

            eps_dma(2)
            eps_dma(3)

            # xT[i, b] bf16 ; layout [128p(i in chunk k), k, b]
            x_nat = persist.tile([BL, F], FP32)
            nc.sync.dma_start(out=x_nat, in_=x_d[:])
            x_bf = persist.tile([BL, F], BF16)
            nc.vector.tensor_copy(x_bf, x_nat)
            xT = persist.tile([128, NCH, BL], BF16)
            for k in range(NCH):
                ptx = ptfp.tile([128, F], BF16, tag="ptf", name=f"ptx{k}")
                nc.tensor.transpose(
                    out=ptx[:, :BL],
                    in_=x_bf[:, ts(k, 128)],
                    identity=ident[:BL, :BL],
                )
                nc.scalar.copy(out=xT[:, k, :], in_=ptx[:, :BL])

            # C (f-ordered) = y_mu + bias_mu + softplus(bias_rho) * eps_b
            bmu_b = persist.tile([BL, F], FP32)
            nc.gpsimd.dma_start(
                out=bmu_b,
                in_=bass.AP(tensor=bmu_d, offset=0, ap=[[0, BL], [1, F]]),
            )
            sb_b = persist.tile([BL, F], FP32)
            nc.gpsimd.dma_start(
                out=sb_b,
                in_=bass.AP(tensor=brho_d, offset=0, ap=[[0, BL], [1, F]]),
            )
            nc.scalar.activation(out=sb_b, in_=sb_b, func=AF.Exp)
            nc.scalar.activation(out=sb_b, in_=sb_b, func=AF.Ln, bias=1.0)
            epsb_s = persist.tile([BL, F], FP32)
            nc.sync.dma_start(out=epsb_s, in_=epsb_d[:])

            nc.vector.tensor_mul(sb_b, sb_b, epsb_s)
            nc.vector.tensor_add(sb_b, sb_b, bmu_b)
            # f-order it: C[b, f] with f = c*128 + p  <->  o = 8p + c
            C = persist.tile([BL, F], FP32)
            Cn_v = sb_b[:].rearrange("b (p c) -> b p c", p=128)
            for c in range(NCH):
                nc.vector.tensor_copy(C[:, ts(c, 128)], Cn_v[:, :, c])

            # y_mu[b, f] = sum_i x[b,i] mu[o(f),i]
            for h in range(2):
                yp = ptfp.tile([BL, 512], FP32, tag="ptf", name=f"ymu{h}")
                for k in range(NCH):
                    nc.tensor.matmul(
                        out=yp,
                        lhsT=xT[:, k, :],
                        rhs=muT[:, k, ts(h, 512)],
                        start=(k == 0),
                        stop=(k == NCH - 1),
                    )
                nc.vector.tensor_add(C[:, ts(h, 512)], C[:, ts(h, 512)], yp)

            # ---------------- main loop over samples ----------------
            for b in range(BL):
                eps_dma(b)          # no-op unless BL < 4 (tiny sim runs)
                eps_dma(b + 4)
                eb = eps_tiles.pop(b)

                # u = eps (*) sigma, one 2x-mode DVE op over all 8 chunks
                u = up.tile([128, NCH, F], BF16, tag="u", name=f"u{b}")
                nc.vector.tensor_mul(u, eb, sig)

                y2 = [
                    py2p.tile([1, 512], FP32, tag=f"y2_{h}", name=f"y2_{h}")
                    for h in range(2)
                ]
                for k in range(NCH):
                    pt_k = ptp.tile([128, F], BF16, tag="pt_k", name=f"pt{b}_{k}")
                    for c in range(NCH):
                        nc.tensor.transpose(
                            out=pt_k[:, ts(c, 128)],
                            in_=u[:, c, ts(k, 128)],
                            identity=ident,
                        )
                    ut_k = utp.tile([128, F], BF16, tag="ut", name=f"ut{b}_{k}")
                    if k < evac_dve:
                        nc.vector.tensor_copy(ut_k, pt_k)
                    else:
                        nc.scalar.copy(out=ut_k, in_=pt_k)
                    for h in range(2):
                        nc.tensor.matmul(
                            out=y2[h],
                            lhsT=xT[:, k, b : b + 1],
                            rhs=ut_k[:, ts(h, 512)],
                            start=(k == 0),
                            stop=(k == NCH - 1),
                        )

                yrow = yrowp.tile([1, F], FP32)
                for h in range(2):
                    nc.scalar.copy(out=yrow[:, ts(h, 512)], in_=y2[h])
                nc.gpsimd.dma_start(
                    out=C[b : b + 1, :], in_=yrow, accum_op=mybir.AluOpType.add
                )

            # undo the f-order permutation and store
            yout = persist.tile([BL, F], FP32)
            yout_v = yout[:].rearrange("b (p c) -> b p c", p=128)
            for c in range(NCH):
                nc.vector.tensor_copy(yout_v[:, :, c], C[:, ts(c, 128)])
            nc.sync.dma_start(out=y_d[:], in_=yout)

    nc.compile()
    return nc


_NC_CACHE: dict[int, bass.Bass] = {}

# overridable build options (used by A/B experiment runners)
BUILD_KWARGS: dict = {}


def _get_nc(BL: int) -> bass.Bass:
    if BL not in _NC_CACHE:
        _NC_CACHE[BL] = build_nc(BL, **BUILD_KWARGS)
    return _NC_CACHE[BL]


def kernel(x, weight_mu, weight_rho, bias_mu, bias_rho, eps_w, eps_b):
    B = x.shape[0]
    BL = B // N_CORES
    nc = _get_nc(BL)

    x = np.ascontiguousarray(np.asarray(x, dtype=np.float32))
    weight_mu = np.ascontiguousarray(np.asarray(weight_mu, dtype=np.float32))
    weight_rho = np.ascontiguousarray(np.asarray(weight_rho, dtype=np.float32))
    bias_mu = np.ascontiguousarray(np.asarray(bias_mu, dtype=np.float32))
    bias_rho = np.ascontiguousarray(np.asarray(bias_rho, dtype=np.float32))
    eps_w = np.ascontiguousarray(np.asarray(eps_w, dtype=np.float32))
    eps_b = np.ascontiguousarray(np.asarray(eps_b, dtype=np.float32))

    in_maps = []
    for i in range(N_CORES):
        sl = slice(i * BL, (i + 1) * BL)
        in_maps.append(
            {
                "x": x[sl],
                "weight_mu": weight_mu,
                "weight_rho": weight_rho,
                "bias_mu": bias_mu,
                "bias_rho": bias_rho,
                "eps_w": eps_w[sl],
                "eps_b": eps_b[sl],
            }
        )

    res = run_bass_kernel_spmd(nc, in_maps, core_ids=list(range(N_CORES)))
    return np.concatenate([r["y"] for r in res.results], axis=0)



# revision 1
# speedup vs baseline: 1.4517x; 1.4517x over previous
"""Bayesian linear layer (reparameterized per-sample weights) on 8 trn2 NeuronCores.

y[b,o] = sum_i x[b,i] * (mu[o,i] + softplus(rho[o,i]) * eps_w[b,o,i])
         + bias_mu[o] + softplus(bias_rho[o]) * eps_b[b,o]

Sharding: data-parallel over batch. 8 cores x 32 samples. mu/rho replicated.

Per-core pipeline (v3-final).  The 128 MiB eps_w shard read dominates
(~426 GB/s measured on the SWDGE cast path); all big loads ride ONE SWDGE
queue in FIFO order (a parallel queue steals HBM read bandwidth from the
eps stream -- measured), and everything else hides under it:
  1. SWDGE DMA casts eps_w[b] fp32->bf16 on the way in, "(p c)" o-layout so
     each partition reads one contiguous 32 KiB run per sample.
  2. DVE single 2x-mode pass u = eps (*) sigma in natural layout (all-SBUF
     bf16 -> 2 elem/cycle/lane).
  3. PE transposes u in bf16 (transpose-mode; pair cost is LDWEIGHTS-bound
     at ~107 ns) into per-k [128, 1024] bf16 PSUM tiles (1 bank each).
  4. Act (5/8) + DVE (3/8) evacuate PSUM -> SBUF bf16 per k-chunk (bf16
     PSUM keeps the DVE copy in 2x mode).
  5. PE reduce-matmul, stationary = x[b, i_chunk] bf16 column (m=1),
     accumulates y2[b, half] over the 8 i-chunks in PSUM.
  6. y_mu + bias terms precomputed into C (f-ordered); per-sample y2 rows
     added via SBUF->SBUF accumulating DMA; one final unpermute + store.
No barriers: eps streaming starts at t=0 and setup hides under it.

Measured on 8xtrn2 (max over cores): 450 us with y2 double-buffered
(pt_bufs=3/ptf=1/y2_bufs=2; the single-buffered variant measured 477-482
back-to-back, 411 in an earlier favorable device state) vs 537 us for the
fp32 transpose-mode baseline; HBM-stream floor is ~340 us.
"""

import numpy as np

import concourse.bass as bass
from concourse import bacc
import concourse.mybir as mybir
import concourse.tile as tile
from concourse.bass import ts
from concourse.bass_utils import run_bass_kernel_spmd
from concourse.masks import make_identity

FP32 = mybir.dt.float32
BF16 = mybir.dt.bfloat16
AF = mybir.ActivationFunctionType

F = 1024          # feature dim (in == out)
N_CORES = 8
B_FULL = 256
NCH = F // 128    # 8 chunks of 128


def build_nc(BL: int, eps_bufs=5, u_bufs=2, ut_bufs=4, pt_bufs=3, y2_bufs=2,
             evac_dve=3) -> bass.Bass:
    """Build the per-core Bass program for a local batch of BL samples.

    o-index layout: o = 8*p + c (partition p in 0..127, chunk c in 0..7), so
    a partition's 8 o-rows are contiguous in HBM.  Column order after the PE
    transpose ("f-order"): f = c*128 + p.  C and y2 are kept f-ordered until
    a single strided pass at the end restores natural o order.
    """
    nc = bacc.Bacc(None, target_bir_lowering=False)

    x_d = nc.declare_dram_parameter("x", [BL, F], FP32, isOutput=False)
    mu_d = nc.declare_dram_parameter("weight_mu", [F, F], FP32, isOutput=False)
    rho_d = nc.declare_dram_parameter("weight_rho", [F, F], FP32, isOutput=False)
    bmu_d = nc.declare_dram_parameter("bias_mu", [F], FP32, isOutput=False)
    brho_d = nc.declare_dram_parameter("bias_rho", [F], FP32, isOutput=False)
    epsw_d = nc.declare_dram_parameter("eps_w", [BL, F, F], FP32, isOutput=False)
    epsb_d = nc.declare_dram_parameter("eps_b", [BL, F], FP32, isOutput=False)
    y_d = nc.declare_dram_parameter("y", [BL, F], FP32, isOutput=True)

    # o = 8p + c: partition p covers o in [8p, 8p+8) -> 32 KiB contiguous.
    epsw_t = epsw_d[:].rearrange("b (p c) i -> b p c i", p=128)
    mu_t = mu_d[:].rearrange("(p c) i -> p c i", p=128)
    rho_t = rho_d[:].rearrange("(p c) i -> p c i", p=128)

    with tile.TileContext(nc) as tc:
        with (
            tc.tile_pool(name="persist", bufs=1) as persist,
            tc.tile_pool(name="setup", bufs=1) as setupp,
            tc.tile_pool(name="eps", bufs=eps_bufs) as epsp,
            tc.tile_pool(name="u", bufs=u_bufs) as up,
            tc.tile_pool(name="ut", bufs=ut_bufs) as utp,
            tc.tile_pool(name="yrow", bufs=2) as yrowp,
            tc.tile_pool(name="pt", bufs=pt_bufs, space="PSUM") as ptp,
            tc.tile_pool(name="ptf", bufs=1, space="PSUM") as ptfp,
            tc.tile_pool(name="py2", bufs=y2_bufs, space="PSUM") as py2p,
        ):
            # ---------------- setup (overlaps with eps streaming) ----------
            ident = persist.tile([128, 128], BF16)
            make_identity(nc, ident)

            # sigma in natural (p c) layout, bf16
            rho_s = setupp.tile([128, NCH, F], BF16, tag="stage", name="rho_s")
            nc.gpsimd.dma_start(out=rho_s, in_=rho_t)
            sig = persist.tile([128, NCH, F], BF16)
            # softplus(x) = ln(1 + exp(x)); rho <= ~0 so no overflow
            nc.scalar.activation(out=sig, in_=rho_s, func=AF.Exp)
            nc.scalar.activation(out=sig, in_=sig, func=AF.Ln, bias=1.0)

            # first eps DMAs issue here (program order on the SWDGE queue:
            # rho, then eps[0..1], then mu, ...)
            eps_tiles: dict[int, object] = {}

            def eps_dma(b):
                if b >= BL or b in eps_tiles:
                    return
                eb = epsp.tile([128, NCH, F], BF16, tag="epst", name=f"eb{b}")
                nc.gpsimd.dma_start(out=eb, in_=epsw_t[b])
                eps_tiles[b] = eb

            eps_dma(0)
            eps_dma(1)

            # muT (bf16): stage in (p c) layout, transpose on PE, evac via Act
            mu_s = setupp.tile([128, NCH, F], BF16, tag="stage", name="mu_s")
            nc.gpsimd.dma_start(out=mu_s, in_=mu_t)
            muT = persist.tile([128, NCH, F], BF16)
            for k in range(NCH):
                pt_k = ptfp.tile([128, F], BF16, tag="ptf", name=f"ptmu{k}")
                for c in range(NCH):
                    nc.tensor.transpose(
                        out=pt_k[:, ts(c, 128)],
                        in_=mu_s[:, c, ts(k, 128)],
                        identity=ident,
                    )
                nc.scalar.copy(out=muT[:, k, :], in_=pt_k)

            eps_dma(2)
            eps_dma(3)

            # xT[i, b] bf16 ; layout [128p(i in chunk k), k, b]
            x_nat = persist.tile([BL, F], FP32)
            nc.sync.dma_start(out=x_nat, in_=x_d[:])
            x_bf = persist.tile([BL, F], BF16)
            nc.vector.tensor_copy(x_bf, x_nat)
            xT = persist.tile([128, NCH, BL], BF16)
            for k in range(NCH):
                ptx = ptfp.tile([128, F], BF16, tag="ptf", name=f"ptx{k}")
                nc.tensor.transpose(
                    out=ptx[:, :BL],
                    in_=x_bf[:, ts(k, 128)],
                    identity=ident[:BL, :BL],
                )
                nc.scalar.copy(out=xT[:, k, :], in_=ptx[:, :BL])

            # C (f-ordered) = y_mu + bias_mu + softplus(bias_rho) * eps_b
            bmu_b = persist.tile([BL, F], FP32)
            nc.gpsimd.dma_start(
                out=bmu_b,
                in_=bass.AP(tensor=bmu_d, offset=0, ap=[[0, BL], [1, F]]),
            )
            sb_b = persist.tile([BL, F], FP32)
            nc.gpsimd.dma_start(
                out=sb_b,
                in_=bass.AP(tensor=brho_d, offset=0, ap=[[0, BL], [1, F]]),
            )
            nc.scalar.activation(out=sb_b, in_=sb_b, func=AF.Exp)
            nc.scalar.activation(out=sb_b, in_=sb_b, func=AF.Ln, bias=1.0)
            epsb_s = persist.tile([BL, F], FP32)
            nc.sync.dma_start(out=epsb_s, in_=epsb_d[:])

            nc.vector.tensor_mul(sb_b, sb_b, epsb_s)
            nc.vector.tensor_add(sb_b, sb_b, bmu_b)
            # f-order it: C[b, f] with f = c*128 + p  <->  o = 8p + c
            C = persist.tile([BL, F], FP32)
            Cn_v = sb_b[:].rearrange("b (p c) -> b p c", p=128)
            for c in range(NCH):
                nc.vector.tensor_copy(C[:, ts(c, 128)], Cn_v[:, :, c])

            # y_mu[b, f] = sum_i x[b,i] mu[o(f),i]
            for h in range(2):
                yp = ptfp.tile([BL, 512], FP32, tag="ptf", name=f"ymu{h}")
                for k in range(NCH):
                    nc.tensor.matmul(
                        out=yp,
                        lhsT=xT[:, k, :],
                        rhs=muT[:, k, ts(h, 512)],
                        start=(k == 0),
                        stop=(k == NCH - 1),
                    )
                nc.vector.tensor_add(C[:, ts(h, 512)], C[:, ts(h, 512)], yp)

            # ---------------- main loop over samples ----------------
            for b in range(BL):
                eps_dma(b)          # no-op unless BL < 4 (tiny sim runs)
                eps_dma(b + 4)
                eb = eps_tiles.pop(b)

                # u = eps (*) sigma, one 2x-mode DVE op over all 8 chunks
                u = up.tile([128, NCH, F], BF16, tag="u", name=f"u{b}")
                nc.vector.tensor_mul(u, eb, sig)

                y2 = [
                    py2p.tile([1, 512], FP32, tag=f"y2_{h}", name=f"y2_{h}")
                    for h in range(2)
                ]
                for k in range(NCH):
                    pt_k = ptp.tile([128, F], BF16, tag="pt_k", name=f"pt{b}_{k}")
                    for c in range(NCH):
                        nc.tensor.transpose(
                            out=pt_k[:, ts(c, 128)],
                            in_=u[:, c, ts(k, 128)],
                            identity=ident,
                        )
                    ut_k = utp.tile([128, F], BF16, tag="ut", name=f"ut{b}_{k}")
                    if k < evac_dve:
                        nc.vector.tensor_copy(ut_k, pt_k)
                    else:
                        nc.scalar.copy(out=ut_k, in_=pt_k)
                    for h in range(2):
                        nc.tensor.matmul(
                            out=y2[h],
                            lhsT=xT[:, k, b : b + 1],
                            rhs=ut_k[:, ts(h, 512)],
                            start=(k == 0),
                            stop=(k == NCH - 1),
                        )

                yrow = yrowp.tile([1, F], FP32)
                for h in range(2):
                    nc.scalar.copy(out=yrow[:, ts(h, 512)], in_=y2[h])
                nc.gpsimd.dma_start(
                    out=C[b : b + 1, :], in_=yrow, accum_op=mybir.AluOpType.add
                )

            # undo the f-order permutation and store
            yout = persist.tile([BL, F], FP32)
            yout_v = yout[:].rearrange("b (p c) -> b p c", p=128)
            for c in range(NCH):
                nc.vector.tensor_copy(yout_v[:, :, c], C[:, ts(c, 128)])
            nc.sync.dma_start(out=y_d[:], in_=yout)

    nc.compile()
    return nc


_NC_CACHE: dict[int, bass.Bass] = {}

# overridable build options (used by A/B experiment runners)
BUILD_KWARGS: dict = {}


def _get_nc(BL: int) -> bass.Bass:
    if BL not in _NC_CACHE:
        _NC_CACHE[BL] = build_nc(BL, **BUILD_KWARGS)
    return _NC_CACHE[BL]


def kernel(x, weight_mu, weight_rho, bias_mu, bias_rho, eps_w, eps_b):
    B = x.shape[0]
    BL = B // N_CORES
    nc = _get_nc(BL)

    x = np.ascontiguousarray(np.asarray(x, dtype=np.float32))
    weight_mu = np.ascontiguousarray(np.asarray(weight_mu, dtype=np.float32))
    weight_rho = np.ascontiguousarray(np.asarray(weight_rho, dtype=np.float32))
    bias_mu = np.ascontiguousarray(np.asarray(bias_mu, dtype=np.float32))
    bias_rho = np.ascontiguousarray(np.asarray(bias_rho, dtype=np.float32))
    eps_w = np.ascontiguousarray(np.asarray(eps_w, dtype=np.float32))
    eps_b = np.ascontiguousarray(np.asarray(eps_b, dtype=np.float32))

    in_maps = []
    for i in range(N_CORES):
        sl = slice(i * BL, (i + 1) * BL)
        in_maps.append(
            {
                "x": x[sl],
                "weight_mu": weight_mu,
                "weight_rho": weight_rho,
                "bias_mu": bias_mu,
                "bias_rho": bias_rho,
                "eps_w": eps_w[sl],
                "eps_b": eps_b[sl],
            }
        )

    res = run_bass_kernel_spmd(nc, in_maps, core_ids=list(range(N_CORES)))
    return np.concatenate([r["y"] for r in res.results], axis=0)

